# revision 1
# baseline (speedup 1.0000x reference)
"""Causal multi-head attention (B=4, T=2048, D=1024, H=16) on 8 trn2 cores.

Sharding: core c -> (batch b = c//2, head-group g = c%2) -> 8 heads/core.
Per-core Bass kernel computes QKV projections, causal flash attention in
transposed-score orientation (s^T = K @ Q^T, softmax denominator via an
appended ones-column in V), and the head-sliced output projection partial.
Host sums the two head-group partials per batch (row-parallel proj).
"""

import numpy as np
import ml_dtypes

import concourse.bass as bass  # noqa: F401  (bass types via bacc)
import concourse.bacc as bacc
import concourse.mybir as mybir
import concourse.tile as tile
from concourse.bass_utils import run_bass_kernel_spmd

B, T, D = 4, 2048, 1024
H, DH = 16, 64
N_CORES = 8
HPC = 8      # heads per core
PAIRS = HPC // 2
BF = mybir.dt.bfloat16
F32 = mybir.dt.float32
BF_NP = ml_dtypes.bfloat16

TQ = 512     # q block (free dim)
TK = 128     # k block (partition dim)
NQG = T // TQ
NKC = T // TK


def build_nc():
    nc = bacc.Bacc(
        "TRN2",
        target_bir_lowering=False,
        debug=False,
        enable_asserts=True,
        num_devices=N_CORES,
    )
    xT = nc.dram_tensor("xT", [D, T], BF, kind="ExternalInput")
    wq = nc.dram_tensor("wq", [D, 512], BF, kind="ExternalInput")
    wk = nc.dram_tensor("wk", [D, 512], BF, kind="ExternalInput")
    wv = nc.dram_tensor("wv", [D, 512], BF, kind="ExternalInput")
    wp = nc.dram_tensor("wp", [512, D], BF, kind="ExternalInput")
    y = nc.dram_tensor("y", [T, D], F32, kind="ExternalOutput")

    with tile.TileContext(nc) as tc:
        with (
            tc.tile_pool(name="pers", bufs=1) as pers,
            tc.tile_pool(name="work", bufs=1) as work,
            tc.tile_pool(name="ps", bufs=1, space="PSUM") as pp,
        ):
            # ---- persistent SBUF ----
            xT_sb = pers.tile([128, 8, T], BF, tag="xT", name="xT_sb")
            wq_sb = pers.tile([128, 8, 512], BF, tag="wq", name="wq_sb")
            wk_sb = pers.tile([128, 8, 512], BF, tag="wk", name="wk_sb")
            wv_sb = pers.tile([128, 8, 512], BF, tag="wv", name="wv_sb")
            wp_sb = pers.tile([128, 4, D], BF, tag="wp", name="wp_sb")
            # V in token-major layout with a ones column per head: [tok, head, 65]
            vext = pers.tile([128, NKC, HPC, 65], BF, tag="vext", name="vext")
            # normalized attention outputs, d-major: [pair-chan, pair, tok]
            outT = pers.tile([128, PAIRS, T], BF, tag="outT", name="outT")
            # causal mask variants for diagonal blocks: keep q >= k + j*128
            mask_sb = pers.tile([128, 128], BF, tag="mask", name="mask_sb")

            # ---- loads ----
            for dc in range(8):
                nc.sync.dma_start(xT_sb[:, dc, :], xT[dc * 128:(dc + 1) * 128, :])
                nc.sync.dma_start(wq_sb[:, dc, :], wq[dc * 128:(dc + 1) * 128, :])
                nc.sync.dma_start(wk_sb[:, dc, :], wk[dc * 128:(dc + 1) * 128, :])
                nc.sync.dma_start(wv_sb[:, dc, :], wv[dc * 128:(dc + 1) * 128, :])
            for cc in range(4):
                nc.sync.dma_start(wp_sb[:, cc, :], wp[cc * 128:(cc + 1) * 128, :])
            nc.gpsimd.memset(vext[:, :, :, 64], 1.0)
            nc.gpsimd.memset(mask_sb[:, :], 1.0)
            nc.gpsimd.affine_select(
                mask_sb[:, :],
                mask_sb[:, :],
                pattern=[[1, 128]],
                compare_op=mybir.AluOpType.is_ge,
                fill=0.0,
                base=0,
                channel_multiplier=-1,
            )

            # ---- phase 1: V = x @ wv  (token-major, all heads at once) ----
            for tk in range(NKC):
                ps_v = pp.tile([128, 512], F32, tag="accQ", bufs=2, name="ps_v")
                for dc in range(8):
                    nc.tensor.matmul(
                        ps_v[:, :],
                        xT_sb[:, dc, tk * 128:(tk + 1) * 128],
                        wv_sb[:, dc, :],
                        start=(dc == 0),
                        stop=(dc == 7),
                    )
                nc.vector.tensor_copy(
                    vext[:, tk, :, 0:64],
                    ps_v.rearrange("p (h d) -> p h d", d=64),
                )

            # ---- phase 2: per head pair ----
            # QT/KT for q-group qg is produced just before the attention that
            # first needs it; the normalize chain of pair p is emitted inside
            # pair p+1's attention so its DVE/GPSIMD burst never blocks the
            # PE at a pair boundary.
            pending_norm = [None]

            def emit_norm():
                if pending_norm[0] is None:
                    return
                hp_, den_, outU_ = pending_norm[0]
                pending_norm[0] = None
                den_r = work.tile([128, 1024], F32, tag="denr", bufs=2,
                                  name="den_r")
                nc.vector.reciprocal(den_r[:, :], den_[:, :])
                for qg_ in range(NQG):
                    for h_ in (0, 1):
                        # partition_broadcast only reads base partition 0 on
                        # HW: stage the reciprocal row through partition 0
                        rc = work.tile([1, 512], F32, tag="rc", bufs=3,
                                       name="rc")
                        nc.vector.tensor_copy(
                            rc[0:1, :],
                            den_r[32 * qg_:32 * qg_ + 1,
                                  h_ * 512:(h_ + 1) * 512],
                        )
                        bc = work.tile([64, 512], F32, tag="bc", bufs=3,
                                       name="bc")
                        nc.gpsimd.partition_broadcast(bc[0:64, :], rc[0:1, :])
                        nc.vector.tensor_mul(
                            outT[h_ * 64:(h_ + 1) * 64, hp_,
                                 qg_ * TQ:(qg_ + 1) * TQ],
                            outU_[(qg_, h_)][0:64, :],
                            bc[0:64, :],
                        )

            for hp in range(PAIRS):
                qt = work.tile([128, T], BF, tag="qt", bufs=2, name="qt")
                kt = work.tile([128, T], BF, tag="kt", bufs=2, name="kt")
                den = work.tile([128, 1024], F32, tag="den", bufs=2, name="den")
                nc.gpsimd.memset(den[:, :], 1.0)
                outU = {}
                for qg in range(NQG):
                    # Q^T / K^T for this q-group, d-major
                    # (rows = pair channels: head0 0-63, head1 64-127)
                    ps_q = pp.tile([128, 512], F32, tag="accQ", bufs=2, name="ps_q")
                    ps_k = pp.tile([128, 512], F32, tag="accQ", bufs=2, name="ps_k")
                    for dc in range(8):
                        nc.tensor.matmul(
                            ps_q[:, :],
                            wq_sb[:, dc, hp * 128:(hp + 1) * 128],
                            xT_sb[:, dc, qg * TQ:(qg + 1) * TQ],
                            start=(dc == 0),
                            stop=(dc == 7),
                        )
                    for dc in range(8):
                        nc.tensor.matmul(
                            ps_k[:, :],
                            wk_sb[:, dc, hp * 128:(hp + 1) * 128],
                            xT_sb[:, dc, qg * TQ:(qg + 1) * TQ],
                            start=(dc == 0),
                            stop=(dc == 7),
                        )
                    nc.vector.tensor_copy(qt[:, qg * TQ:(qg + 1) * TQ], ps_q[:, :])
                    nc.vector.tensor_copy(kt[:, qg * TQ:(qg + 1) * TQ], ps_k[:, :])

                    # attention over k chunks 0..(qg+1)*4, software-pipelined:
                    # QK of chunk kc+1 is emitted before AV of chunk kc
                    psO0 = pp.tile([65, 512], F32, tag="accO", bufs=2, name="psO0")
                    psO1 = pp.tile([65, 512], F32, tag="accO", bufs=2, name="psO1")
                    kmax = (qg + 1) * (TQ // TK)
                    noff = qg * (TQ // TK)

                    def qk(kc):
                        # scores^T chunk for both heads: [k 128, q 512] x2
                        # on diagonal blocks only columns q >= j*128 are live
                        off = max(0, kc - noff) * TK
                        ps_s = pp.tile([128, 1024], F32, tag="sc", bufs=2, name="ps_s")
                        for h in (0, 1):
                            nc.tensor.matmul(
                                ps_s[:, h * 512 + off:(h + 1) * 512],
                                kt[h * 64:(h + 1) * 64, kc * TK:(kc + 1) * TK],
                                qt[h * 64:(h + 1) * 64, qg * TQ + off:(qg + 1) * TQ],
                                start=True, stop=True,
                            )
                        return ps_s

                    def softmax_av(kc, ps_s):
                        off = max(0, kc - noff) * TK
                        j = kc - noff
                        ex = work.tile([128, 1024], BF, tag="ex", bufs=6, name="ex")
                        for h, psO in ((0, psO0), (1, psO1)):
                            sl = slice(h * 512 + off, (h + 1) * 512)
                            nc.scalar.activation(
                                ex[:, sl], ps_s[:, sl],
                                mybir.ActivationFunctionType.Exp,
                            )
                            if j >= 0:
                                # causal mask on the diagonal 128x128 sub-block
                                msl = slice(h * 512 + off, h * 512 + off + TK)
                                nc.vector.tensor_mul(
                                    ex[:, msl], ex[:, msl], mask_sb[:, :]
                                )
                            nc.tensor.matmul(
                                psO[:, off:512],
                                vext[:, kc, hp * 2 + h, :],
                                ex[:, sl],
                                start=(kc == 0),
                                stop=(kc == kmax - 1),
                                skip_group_check=True,
                            )

                    prev = qk(0)
                    for kc in range(kmax):
                        nxt = qk(kc + 1) if kc + 1 < kmax else None
                        softmax_av(kc, prev)
                        prev = nxt

                    # evict unnormalized AV + denominator row to SBUF,
                    # freeing PSUM; stash denom rows for the batched recip
                    for h, psO in ((0, psO0), (1, psO1)):
                        oU = work.tile([65, 512], F32, tag="outU", bufs=16,
                                       name="oU")
                        nc.vector.tensor_copy(oU[:, :], psO[:, :])
                        nc.vector.tensor_copy(
                            den[32 * qg:32 * qg + 1, h * 512:(h + 1) * 512],
                            psO[64:65, :],
                        )
                        outU[(qg, h)] = oU

                    if qg == 0:
                        # previous pair's normalize lands here, overlapped
                        # with this pair's remaining attention
                        emit_norm()

                pending_norm[0] = (hp, den, outU)

            emit_norm()

            # ---- phase 3: y_partial = outT.T @ wp ----
            for tk in range(NKC):
                for nb in range(2):
                    ps_y = pp.tile([128, 512], F32, tag="accQ", bufs=2, name="ps_y")
                    for cc in range(4):
                        nc.tensor.matmul(
                            ps_y[:, :],
                            outT[:, cc, tk * 128:(tk + 1) * 128],
                            wp_sb[:, cc, nb * 512:(nb + 1) * 512],
                            start=(cc == 0),
                            stop=(cc == 3),
                        )
                    y_ev = work.tile([128, 512], F32, tag="yev", bufs=3, name="y_ev")
                    nc.scalar.copy(y_ev[:, :], ps_y[:, :])
                    nc.sync.dma_start(
                        y[tk * 128:(tk + 1) * 128, nb * 512:(nb + 1) * 512],
                        y_ev[:, :],
                    )

    nc.compile()
    return nc


_NC_CACHE = None


def _get_nc():
    global _NC_CACHE
    if _NC_CACHE is None:
        _NC_CACHE = build_nc()
    return _NC_CACHE


def make_in_maps(x, w_qkv, w_proj):
    """Host-side sharding: core c -> (batch c//2, head-group c%2)."""
    scale = np.float32(1.0 / np.sqrt(DH))
    in_maps = []
    for c in range(N_CORES):
        b, g = divmod(c, 2)
        sl = slice(g * 512, (g + 1) * 512)
        xT = np.ascontiguousarray(x[b].T).astype(BF_NP)
        wq = (w_qkv[:, 0 * D:1 * D][:, sl] * scale).astype(BF_NP)
        wk = w_qkv[:, 1 * D:2 * D][:, sl].astype(BF_NP)
        wv = w_qkv[:, 2 * D:3 * D][:, sl].astype(BF_NP)
        wp = np.ascontiguousarray(w_proj[sl, :]).astype(BF_NP)
        in_maps.append({"xT": xT, "wq": wq, "wk": wk, "wv": wv, "wp": wp})
    return in_maps


def kernel(x, w_qkv, w_proj, _trace=False, _tmpdir=None):
    x = np.asarray(x, dtype=np.float32)
    w_qkv = np.asarray(w_qkv, dtype=np.float32)
    w_proj = np.asarray(w_proj, dtype=np.float32)
    nc = _get_nc()
    in_maps = make_in_maps(x, w_qkv, w_proj)
    res = run_bass_kernel_spmd(
        nc, in_maps, core_ids=list(range(N_CORES)), trace=_trace, tmpdir=_tmpdir
    )
    out = np.empty((B, T, D), dtype=np.float32)
    for b in range(B):
        out[b] = res.results[2 * b]["y"] + res.results[2 * b + 1]["y"]
    if _trace:
        kernel._last_results = res
    return out



# revision 3
# speedup vs baseline: 1.1355x; 1.1355x over previous
"""Causal multi-head attention (B=4, T=2048, D=1024, H=16) on 8 trn2 cores.

Sharding: core c -> (batch b = c//2, head-group g = c%2) -> 8 heads/core.

Per-core schedule (ACT-engine/exp is the roofline):
  - qg-outer / pair-inner loops; QKV projections and the output projection
    are emitted as fine-grained "filler" interleaved into the attention
    chunk stream so the PE works while ACT (exp) is saturated.
  - exp is one merged two-head instruction per k-chunk [128, 2, 512-off].
  - V carries 64 ones-columns so the AV matmul materializes the softmax
    denominator replicated across PSUM partitions 64..127; normalization is
    a single reciprocal_approx_fast + two fused multiply-evictions on DVE.
  - causal mask applied by gpsimd affine_select directly on ex.
"""

import numpy as np
import ml_dtypes

import concourse.bass as bass  # noqa: F401  (bass types via bacc)
import concourse.bacc as bacc
import concourse.mybir as mybir
import concourse.tile as tile
from concourse.bass_utils import run_bass_kernel_spmd

B, T, D = 4, 2048, 1024
H, DH = 16, 64
N_CORES = 8
HPC = 8      # heads per core
PAIRS = HPC // 2
BF = mybir.dt.bfloat16
F32 = mybir.dt.float32
BF_NP = ml_dtypes.bfloat16

TQ = 512     # q block (free dim)
TK = 128     # k block (partition dim)
NQG = T // TQ
NKC = T // TK


def build_nc():
    nc = bacc.Bacc(
        "TRN2",
        target_bir_lowering=False,
        debug=False,
        enable_asserts=True,
        num_devices=N_CORES,
    )
    xT = nc.dram_tensor("xT", [D, T], BF, kind="ExternalInput")
    wq = nc.dram_tensor("wq", [D, 512], BF, kind="ExternalInput")
    wk = nc.dram_tensor("wk", [D, 512], BF, kind="ExternalInput")
    wv = nc.dram_tensor("wv", [D, 512], BF, kind="ExternalInput")
    wp = nc.dram_tensor("wp", [512, D], BF, kind="ExternalInput")
    y = nc.dram_tensor("y", [T, D], F32, kind="ExternalOutput")

    with tile.TileContext(nc) as tc:
        with (
            tc.tile_pool(name="pers", bufs=1) as pers,
            tc.tile_pool(name="work", bufs=1) as work,
            tc.tile_pool(name="ps", bufs=1, space="PSUM") as pp,
        ):
            # ---- persistent SBUF ----
            xT_sb = pers.tile([128, 8, T], BF, tag="xT", name="xT_sb")
            wq_sb = pers.tile([128, 8, 512], BF, tag="wq", name="wq_sb")
            wk_sb = pers.tile([128, 8, 512], BF, tag="wk", name="wk_sb")
            wv_sb = pers.tile([128, 8, 512], BF, tag="wv", name="wv_sb")
            wp_sb = pers.tile([128, 4, D], BF, tag="wp", name="wp_sb")
            # V in token-major layout; cols 64..127 are ones so the AV
            # matmul writes the denominator to psO partitions 64..127.
            vext = pers.tile([128, NKC, HPC, 128], BF, tag="vext", name="vext")
            qt_all = pers.tile([128, PAIRS, T], BF, tag="qt", name="qt_all")
            kt_all = pers.tile([128, PAIRS, T], BF, tag="kt", name="kt_all")
            # normalized attention outputs, d-major: [pair-chan, pair, tok]
            outT = pers.tile([128, PAIRS, T], BF, tag="outT", name="outT")

            # ---- loads (ordered so qg0/pair0 deps land first) ----
            for dc in range(8):
                nc.sync.dma_start(wq_sb[:, dc, :], wq[dc * 128:(dc + 1) * 128, :])
                nc.sync.dma_start(wk_sb[:, dc, :], wk[dc * 128:(dc + 1) * 128, :])
                nc.sync.dma_start(wv_sb[:, dc, :], wv[dc * 128:(dc + 1) * 128, :])
                nc.sync.dma_start(xT_sb[:, dc, 0:512], xT[dc * 128:(dc + 1) * 128, 0:512])
            for dc in range(8):
                nc.sync.dma_start(xT_sb[:, dc, 512:T], xT[dc * 128:(dc + 1) * 128, 512:T])
            for cc in range(4):
                nc.sync.dma_start(wp_sb[:, cc, :], wp[cc * 128:(cc + 1) * 128, :])
            nc.gpsimd.memset(vext[:, :, :, 64:128], 1.0)

            # ---- filler machinery ----
            filler = []
            fptr = [0]

            def pump(n=1):
                while n > 0 and fptr[0] < len(filler):
                    filler[fptr[0]]()
                    fptr[0] += 1
                    n -= 1

            def drain_until(idx):
                while fptr[0] <= idx:
                    if fptr[0] >= len(filler):
                        return
                    filler[fptr[0]]()
                    fptr[0] += 1

            # V tile t covers token chunks 2t, 2t+1 (all heads)
            def v_tile_closures(t):
                st = {}

                def mm(half, dlo):
                    if half == 0 and dlo == 0:
                        st["ps"] = pp.tile([128, 1024], F32, tag="sc", bufs=2,
                                           name="ps_v")
                    tk = 2 * t + half
                    for dc in range(dlo, dlo + 4):
                        nc.tensor.matmul(
                            st["ps"][:, half * 512:(half + 1) * 512],
                            xT_sb[:, dc, tk * 128:(tk + 1) * 128],
                            wv_sb[:, dc, :],
                            start=(dc == 0), stop=(dc == 7),
                        )

                def cast():
                    nc.vector.tensor_copy(
                        vext[:, 2 * t:2 * t + 2, :, 0:64],
                        st["ps"].rearrange("p (t h d) -> p t h d", t=2, d=64),
                    )

                return [lambda h=h, d=d: mm(h, d) for h in (0, 1) for d in (0, 4)] + [cast]

            # QK^T projection for (qg, pair): d-major Q^T/K^T of 512 tokens
            def qkt_closures(qg, pair):
                st = {}
                sl = slice(qg * TQ, (qg + 1) * TQ)

                def mm(w_sb, half, dlo):
                    if half == 0 and dlo == 0:
                        st["ps"] = pp.tile([128, 1024], F32, tag="sc", bufs=2,
                                           name="ps_qk")
                    for dc in range(dlo, dlo + 4):
                        nc.tensor.matmul(
                            st["ps"][:, half * 512:(half + 1) * 512],
                            w_sb[:, dc, pair * 128:(pair + 1) * 128],
                            xT_sb[:, dc, sl],
                            start=(dc == 0), stop=(dc == 7),
                        )

                def cast():
                    nc.vector.tensor_copy(qt_all[:, pair, sl], st["ps"][:, 0:512])
                    nc.vector.tensor_copy(kt_all[:, pair, sl], st["ps"][:, 512:1024])

                return [
                    lambda d=0: mm(wq_sb, 0, d),
                    lambda d=4: mm(wq_sb, 0, d),
                    lambda d=0: mm(wk_sb, 1, d),
                    lambda d=4: mm(wk_sb, 1, d),
                    cast,
                ]

            # output projection for one token chunk tk (needs outT of all pairs)
            def proj_closures(tk):
                st = {}

                def mm(nb):
                    if nb == 0:
                        st["ps"] = pp.tile([128, 1024], F32, tag="sc", bufs=2,
                                           name="ps_y")
                    for cc in range(4):
                        nc.tensor.matmul(
                            st["ps"][:, nb * 512:(nb + 1) * 512],
                            outT[:, cc, tk * 128:(tk + 1) * 128],
                            wp_sb[:, cc, nb * 512:(nb + 1) * 512],
                            start=(cc == 0), stop=(cc == 3),
                        )

                def evict():
                    y_sb = work.tile([128, 1024], F32, tag="ysb", bufs=2,
                                     name="y_sb")
                    nc.vector.tensor_copy(y_sb[:, :], st["ps"][:, :])
                    nc.sync.dma_start(y[tk * 128:(tk + 1) * 128, :], y_sb[:, :])

                return [lambda: mm(0), lambda: mm(1), evict]

            # prefix: qg0/pair0 QK^T and V chunks 0..3 emitted directly
            for cl in qkt_closures(0, 0):
                cl()
            for t in (0, 1):
                for cl in v_tile_closures(t):
                    cl()

            # filler queue + markers: marker[(qg, pair)] = last filler index
            # that must be emitted before attention (qg, pair) starts
            marker = {(0, 0): -1}
            for t in (2, 3):
                filler.extend(v_tile_closures(t))
            for p in (1, 2, 3):
                filler.extend(qkt_closures(0, p))
                marker[(0, p)] = len(filler) - 1
            for qg in (1, 2, 3):
                for t in (2 * qg + 2, 2 * qg + 3):
                    if t < 8:
                        filler.extend(v_tile_closures(t))
                for p in range(4):
                    filler.extend(qkt_closures(qg, p))
                    marker[(qg, p)] = len(filler) - 1

            # ---- attention instances ----
            pending_norm = [None]

            def emit_norm():
                if pending_norm[0] is None:
                    return
                psO_, pair_, qg_ = pending_norm[0]
                pending_norm[0] = None
                den_sb = work.tile([64, 1024], F32, tag="den", bufs=2,
                                   name="den_sb")
                bcr = work.tile([64, 1024], F32, tag="bcr", bufs=2, name="bcr")
                nc.vector.tensor_copy(den_sb[:, :], psO_[64:128, :])
                nc.vector.reciprocal_approx_fast(bcr[:, :], den_sb[:, :])
                for h in (0, 1):
                    nc.vector.tensor_mul(
                        outT[h * 64:(h + 1) * 64, pair_,
                             qg_ * TQ:(qg_ + 1) * TQ],
                        psO_[0:64, h * 512:(h + 1) * 512],
                        bcr[0:64, h * 512:(h + 1) * 512],
                    )

            for qg in range(NQG):
                noff = qg * (TQ // TK)
                kmax = (qg + 1) * (TQ // TK)
                for pair in range(PAIRS):
                    drain_until(marker[(qg, pair)])
                    psO = pp.tile([128, 1024], F32, tag="o", bufs=2, name="psO")

                    def qk(kc):
                        off = max(0, kc - noff) * TK
                        ps_s = pp.tile([128, 1024], F32, tag="sc", bufs=2,
                                       name="ps_s")
                        for h in (0, 1):
                            nc.tensor.matmul(
                                ps_s[:, h * 512 + off:(h + 1) * 512],
                                kt_all[h * 64:(h + 1) * 64, pair,
                                       kc * TK:(kc + 1) * TK],
                                qt_all[h * 64:(h + 1) * 64, pair,
                                       qg * TQ + off:(qg + 1) * TQ],
                                start=True, stop=True,
                            )
                        return ps_s

                    def softmax_av(kc, ps_s):
                        off = max(0, kc - noff) * TK
                        ex = work.tile([128, 2, 512], BF, tag="ex", bufs=6,
                                       name="ex")
                        s3 = ps_s.rearrange("p (h q) -> p h q", h=2)
                        nc.scalar.activation(
                            ex[:, :, off:512], s3[:, :, off:512],
                            mybir.ActivationFunctionType.Exp,
                        )
                        if kc >= noff:
                            # causal mask on the diagonal 128-col sub-block
                            nc.gpsimd.affine_select(
                                ex[:, :, off:off + TK],
                                ex[:, :, off:off + TK],
                                pattern=[[0, 2], [1, TK]],
                                compare_op=mybir.AluOpType.is_ge,
                                fill=0.0,
                                base=0,
                                channel_multiplier=-1,
                            )
                        for h in (0, 1):
                            nc.tensor.matmul(
                                psO[:, h * 512 + off:(h + 1) * 512],
                                vext[:, kc, pair * 2 + h, :],
                                ex[:, h, off:512],
                                start=(kc == 0),
                                stop=(kc == kmax - 1),
                                skip_group_check=True,
                            )

                    prev = qk(0)
                    emit_norm()  # previous instance's normalize, overlapped
                    if pair == 0 and qg > 0:
                        for tk in range((qg - 1) * 4, qg * 4):
                            filler.extend(proj_closures(tk))
                    for kc in range(kmax):
                        nxt = qk(kc + 1) if kc + 1 < kmax else None
                        softmax_av(kc, prev)
                        pump(1)
                        prev = nxt

                    pending_norm[0] = (psO, pair, qg)

            # ---- tail: last normalize + last qg's projection ----
            emit_norm()
            for tk in range(12, 16):
                filler.extend(proj_closures(tk))
            drain_until(len(filler) - 1)

    nc.compile()
    return nc


_NC_CACHE = None


def _get_nc():
    global _NC_CACHE
    if _NC_CACHE is None:
        _NC_CACHE = build_nc()
    return _NC_CACHE


def make_in_maps(x, w_qkv, w_proj):
    """Host-side sharding: core c -> (batch c//2, head-group c%2)."""
    scale = np.float32(1.0 / np.sqrt(DH))
    in_maps = []
    for c in range(N_CORES):
        b, g = divmod(c, 2)
        sl = slice(g * 512, (g + 1) * 512)
        xT = np.ascontiguousarray(x[b].T).astype(BF_NP)
        wq = (w_qkv[:, 0 * D:1 * D][:, sl] * scale).astype(BF_NP)
        wk = w_qkv[:, 1 * D:2 * D][:, sl].astype(BF_NP)
        wv = w_qkv[:, 2 * D:3 * D][:, sl].astype(BF_NP)
        wp = np.ascontiguousarray(w_proj[sl, :]).astype(BF_NP)
        in_maps.append({"xT": xT, "wq": wq, "wk": wk, "wv": wv, "wp": wp})
    return in_maps


def kernel(x, w_qkv, w_proj, _trace=False, _tmpdir=None):
    x = np.asarray(x, dtype=np.float32)
    w_qkv = np.asarray(w_qkv, dtype=np.float32)
    w_proj = np.asarray(w_proj, dtype=np.float32)
    nc = _get_nc()
    in_maps = make_in_maps(x, w_qkv, w_proj)
    res = run_bass_kernel_spmd(
        nc, in_maps, core_ids=list(range(N_CORES)), trace=_trace, tmpdir=_tmpdir
    )
    out = np.empty((B, T, D), dtype=np.float32)
    for b in range(B):
        out[b] = res.results[2 * b]["y"] + res.results[2 * b + 1]["y"]
    if _trace:
        kernel._last_results = res
    return out


# revision 5
# speedup vs baseline: 1.1598x; 1.0214x over previous
"""Causal multi-head attention (B=4, T=2048, D=1024, H=16) on 8 trn2 cores.

Sharding: core c -> (batch b = c//2, head-group g = c%2) -> 8 heads/core.

Per-core schedule (ACT-engine/exp is the roofline):
  - qg-outer loops; the 4 head pairs are processed as 2 blocks of TWO
    INTERLEAVED STREAMS so two independent exp->AV->QK dependency chains
    alternate on ACT and hide each other's semaphore latency.
  - QKV projections and the output projection are emitted as fine-grained
    "filler" interleaved into the attention chunk stream.
  - exp is one merged two-head instruction per k-chunk [128, 2, 512-off].
  - V carries 64 ones-columns so the AV matmul materializes the softmax
    denominator replicated across PSUM partitions 64..127; normalization is
    tensor_copy + reciprocal_approx_fast + two fused multiply-evictions.
  - causal mask applied by gpsimd affine_select directly on ex.
"""

import numpy as np
import ml_dtypes

import concourse.bass as bass  # noqa: F401  (bass types via bacc)
import concourse.bacc as bacc
import concourse.mybir as mybir
import concourse.tile as tile
from concourse.bass_utils import run_bass_kernel_spmd

B, T, D = 4, 2048, 1024
H, DH = 16, 64
N_CORES = 8
HPC = 8      # heads per core
PAIRS = HPC // 2
BF = mybir.dt.bfloat16
F32 = mybir.dt.float32
BF_NP = ml_dtypes.bfloat16

TQ = 512     # q block (free dim)
TK = 128     # k block (partition dim)
NQG = T // TQ
NKC = T // TK


def build_nc():
    nc = bacc.Bacc(
        "TRN2",
        target_bir_lowering=False,
        debug=False,
        enable_asserts=True,
        num_devices=N_CORES,
    )
    xT = nc.dram_tensor("xT", [D, T], BF, kind="ExternalInput")
    wq = nc.dram_tensor("wq", [D, 512], BF, kind="ExternalInput")
    wk = nc.dram_tensor("wk", [D, 512], BF, kind="ExternalInput")
    wv = nc.dram_tensor("wv", [D, 512], BF, kind="ExternalInput")
    wp = nc.dram_tensor("wp", [512, D], BF, kind="ExternalInput")
    y = nc.dram_tensor("y", [T, D], F32, kind="ExternalOutput")

    with tile.TileContext(nc) as tc:
        with (
            tc.tile_pool(name="pers", bufs=1) as pers,
            tc.tile_pool(name="work", bufs=1) as work,
            tc.tile_pool(name="ps", bufs=1, space="PSUM") as pp,
        ):
            # ---- persistent SBUF ----
            xT_sb = pers.tile([128, 8, T], BF, tag="xT", name="xT_sb")
            wq_sb = pers.tile([128, 8, 512], BF, tag="wq", name="wq_sb")
            wk_sb = pers.tile([128, 8, 512], BF, tag="wk", name="wk_sb")
            wv_sb = pers.tile([128, 8, 512], BF, tag="wv", name="wv_sb")
            wp_sb = pers.tile([128, 4, D], BF, tag="wp", name="wp_sb")
            # V in token-major layout; cols 64..127 are ones so the AV
            # matmul writes the denominator to psO partitions 64..127.
            vext = pers.tile([128, NKC, HPC, 128], BF, tag="vext", name="vext")
            qt_all = pers.tile([128, PAIRS, T], BF, tag="qt", name="qt_all")
            kt_all = pers.tile([128, PAIRS, T], BF, tag="kt", name="kt_all")
            # normalized attention outputs, d-major: [pair-chan, pair, tok]
            outT = pers.tile([128, PAIRS, T], BF, tag="outT", name="outT")

            # ---- loads (ordered so qg0 pair0/1 deps land first) ----
            for dc in range(8):
                nc.sync.dma_start(wq_sb[:, dc, :], wq[dc * 128:(dc + 1) * 128, :])
                nc.sync.dma_start(wk_sb[:, dc, :], wk[dc * 128:(dc + 1) * 128, :])
                nc.sync.dma_start(wv_sb[:, dc, :], wv[dc * 128:(dc + 1) * 128, :])
                nc.sync.dma_start(xT_sb[:, dc, 0:512], xT[dc * 128:(dc + 1) * 128, 0:512])
            for dc in range(8):
                nc.sync.dma_start(xT_sb[:, dc, 512:T], xT[dc * 128:(dc + 1) * 128, 512:T])
            for cc in range(4):
                nc.sync.dma_start(wp_sb[:, cc, :], wp[cc * 128:(cc + 1) * 128, :])
            nc.gpsimd.memset(vext[:, :, :, 64:128], 1.0)

            # ---- filler machinery ----
            filler = []
            fptr = [0]

            def pump(n=1):
                while n > 0 and fptr[0] < len(filler):
                    filler[fptr[0]]()
                    fptr[0] += 1
                    n -= 1

            def drain_until(idx):
                while fptr[0] <= idx:
                    if fptr[0] >= len(filler):
                        return
                    filler[fptr[0]]()
                    fptr[0] += 1

            # V tile t covers token chunks 2t, 2t+1 (all heads)
            def v_tile_closures(t):
                st = {}

                def mm(half, dlo):
                    if half == 0 and dlo == 0:
                        st["ps"] = pp.tile([128, 1024], F32, tag="sc", bufs=2,
                                           name="ps_v")
                    tk = 2 * t + half
                    for dc in range(dlo, dlo + 4):
                        nc.tensor.matmul(
                            st["ps"][:, half * 512:(half + 1) * 512],
                            xT_sb[:, dc, tk * 128:(tk + 1) * 128],
                            wv_sb[:, dc, :],
                            start=(dc == 0), stop=(dc == 7),
                        )

                def cast():
                    nc.vector.tensor_copy(
                        vext[:, 2 * t:2 * t + 2, :, 0:64],
                        st["ps"].rearrange("p (t h d) -> p t h d", t=2, d=64),
                    )

                return [lambda h=h, d=d: mm(h, d) for h in (0, 1) for d in (0, 4)] + [cast]

            # QK^T projection for (qg, pair): d-major Q^T/K^T of 512 tokens
            def qkt_closures(qg, pair):
                st = {}
                sl = slice(qg * TQ, (qg + 1) * TQ)

                def mm(w_sb, half, dlo):
                    if half == 0 and dlo == 0:
                        st["ps"] = pp.tile([128, 1024], F32, tag="sc", bufs=2,
                                           name="ps_qk")
                    for dc in range(dlo, dlo + 4):
                        nc.tensor.matmul(
                            st["ps"][:, half * 512:(half + 1) * 512],
                            w_sb[:, dc, pair * 128:(pair + 1) * 128],
                            xT_sb[:, dc, sl],
                            start=(dc == 0), stop=(dc == 7),
                        )

                def cast():
                    nc.vector.tensor_copy(qt_all[:, pair, sl], st["ps"][:, 0:512])
                    nc.vector.tensor_copy(kt_all[:, pair, sl], st["ps"][:, 512:1024])

                return [
                    lambda d=0: mm(wq_sb, 0, d),
                    lambda d=4: mm(wq_sb, 0, d),
                    lambda d=0: mm(wk_sb, 1, d),
                    lambda d=4: mm(wk_sb, 1, d),
                    cast,
                ]

            # output projection for one token chunk tk (needs outT of all pairs)
            def proj_closures(tk):
                st = {}

                def mm(nb):
                    if nb == 0:
                        st["ps"] = pp.tile([128, 1024], F32, tag="sc", bufs=2,
                                           name="ps_y")
                    for cc in range(4):
                        nc.tensor.matmul(
                            st["ps"][:, nb * 512:(nb + 1) * 512],
                            outT[:, cc, tk * 128:(tk + 1) * 128],
                            wp_sb[:, cc, nb * 512:(nb + 1) * 512],
                            start=(cc == 0), stop=(cc == 3),
                        )

                def evict():
                    y_sb = work.tile([128, 1024], F32, tag="ysb", bufs=2,
                                     name="y_sb")
                    nc.vector.tensor_copy(y_sb[:, :], st["ps"][:, :])
                    nc.sync.dma_start(y[tk * 128:(tk + 1) * 128, :], y_sb[:, :])

                return [lambda: mm(0), lambda: mm(1), evict]

            # prefix: qg0 pair0+pair1 QK^T and V chunks 0..3 emitted directly
            for p in (0, 1):
                for cl in qkt_closures(0, p):
                    cl()
            for t in (0, 1):
                for cl in v_tile_closures(t):
                    cl()

            # filler queue + markers[(qg,pair)] = last filler index that must
            # be emitted before attention on (qg, pair) starts
            marker = {(0, 0): -1, (0, 1): -1}
            for p in (2, 3):
                filler.extend(qkt_closures(0, p))
                marker[(0, p)] = len(filler) - 1
            for qg in (1, 2, 3):
                for t in (2 * qg, 2 * qg + 1):
                    filler.extend(v_tile_closures(t))
                for p in range(4):
                    filler.extend(qkt_closures(qg, p))
                    marker[(qg, p)] = len(filler) - 1

            # ---- attention: 2 blocks of 2 interleaved pair-streams per qg ----
            pending_mults = []

            def emit_pending_mults():
                while pending_mults:
                    pending_mults.pop(0)()

            for qg in range(NQG):
                noff = qg * (TQ // TK)
                kmax = (qg + 1) * (TQ // TK)

                def qk(pair, kc):
                    off = max(0, kc - noff) * TK
                    ps_s = pp.tile([128, 1024], F32, tag="sc", bufs=2,
                                   name="ps_s")
                    for h in (0, 1):
                        nc.tensor.matmul(
                            ps_s[:, h * 512 + off:(h + 1) * 512],
                            kt_all[h * 64:(h + 1) * 64, pair,
                                   kc * TK:(kc + 1) * TK],
                            qt_all[h * 64:(h + 1) * 64, pair,
                                   qg * TQ + off:(qg + 1) * TQ],
                            start=True, stop=True,
                        )
                    return ps_s

                def softmax_av(pair, kc, ps_s, psO):
                    off = max(0, kc - noff) * TK
                    ex = work.tile([128, 2, 512], BF, tag="ex", bufs=6,
                                   name="ex")
                    s3 = ps_s.rearrange("p (h q) -> p h q", h=2)
                    nc.scalar.activation(
                        ex[:, :, off:512], s3[:, :, off:512],
                        mybir.ActivationFunctionType.Exp,
                    )
                    if kc >= noff:
                        # causal mask on the diagonal 128-col sub-block
                        nc.gpsimd.affine_select(
                            ex[:, :, off:off + TK],
                            ex[:, :, off:off + TK],
                            pattern=[[0, 2], [1, TK]],
                            compare_op=mybir.AluOpType.is_ge,
                            fill=0.0,
                            base=0,
                            channel_multiplier=-1,
                        )
                    for h in (0, 1):
                        nc.tensor.matmul(
                            psO[:, h * 512 + off:(h + 1) * 512],
                            vext[:, kc, pair * 2 + h, :],
                            ex[:, h, off:512],
                            start=(kc == 0),
                            stop=(kc == kmax - 1),
                            skip_group_check=True,
                        )

                def finish_pair(pair, psO, qg=qg):
                    """Emit den copy + recip now; queue the multiply-evicts.

                    qg is bound at definition time: the mults closure runs
                    during the NEXT qg iteration, after the loop var moved on.
                    """
                    den_sb = work.tile([64, 1024], F32, tag="den", bufs=2,
                                       name="den_sb")
                    bcr = work.tile([64, 1024], F32, tag="bcr", bufs=2,
                                    name="bcr")
                    nc.vector.tensor_copy(den_sb[:, :], psO[64:128, :])
                    nc.vector.reciprocal_approx_fast(bcr[:, :], den_sb[:, :])

                    def mults():
                        for h in (0, 1):
                            nc.vector.tensor_mul(
                                outT[h * 64:(h + 1) * 64, pair,
                                     qg * TQ:(qg + 1) * TQ],
                                psO[0:64, h * 512:(h + 1) * 512],
                                bcr[0:64, h * 512:(h + 1) * 512],
                            )

                    pending_mults.append(mults)

                for blk in (0, 1):
                    pA, pB = 2 * blk, 2 * blk + 1
                    drain_until(max(marker[(qg, pA)], marker[(qg, pB)]))
                    psO_A = pp.tile([128, 1024], F32, tag="o", bufs=2,
                                    name="psO_A")
                    psO_B = pp.tile([128, 1024], F32, tag="o", bufs=2,
                                    name="psO_B")
                    sA = qk(pA, 0)
                    sB = qk(pB, 0)
                    emit_pending_mults()
                    if blk == 0 and qg > 0:
                        for tk in range((qg - 1) * 4, qg * 4):
                            filler.extend(proj_closures(tk))
                    for kc in range(kmax):
                        nA = qk(pA, kc + 1) if kc + 1 < kmax else None
                        softmax_av(pA, kc, sA, psO_A)
                        nB = qk(pB, kc + 1) if kc + 1 < kmax else None
                        softmax_av(pB, kc, sB, psO_B)
                        pump(2)
                        sA, sB = nA, nB
                    finish_pair(pA, psO_A)
                    finish_pair(pB, psO_B)

            # ---- tail: last mults + last qg's projection ----
            emit_pending_mults()
            for tk in range(12, 16):
                filler.extend(proj_closures(tk))
            drain_until(len(filler) - 1)

    nc.compile()
    return nc


_NC_CACHE = None


def _get_nc():
    global _NC_CACHE
    if _NC_CACHE is None:
        _NC_CACHE = build_nc()
    return _NC_CACHE


def make_in_maps(x, w_qkv, w_proj):
    """Host-side sharding: core c -> (batch c//2, head-group c%2)."""
    scale = np.float32(1.0 / np.sqrt(DH))
    in_maps = []
    for c in range(N_CORES):
        b, g = divmod(c, 2)
        sl = slice(g * 512, (g + 1) * 512)
        xT = np.ascontiguousarray(x[b].T).astype(BF_NP)
        wq = (w_qkv[:, 0 * D:1 * D][:, sl] * scale).astype(BF_NP)
        wk = w_qkv[:, 1 * D:2 * D][:, sl].astype(BF_NP)
        wv = w_qkv[:, 2 * D:3 * D][:, sl].astype(BF_NP)
        wp = np.ascontiguousarray(w_proj[sl, :]).astype(BF_NP)
        in_maps.append({"xT": xT, "wq": wq, "wk": wk, "wv": wv, "wp": wp})
    return in_maps


def kernel(x, w_qkv, w_proj, _trace=False, _tmpdir=None):
    x = np.asarray(x, dtype=np.float32)
    w_qkv = np.asarray(w_qkv, dtype=np.float32)
    w_proj = np.asarray(w_proj, dtype=np.float32)
    nc = _get_nc()
    in_maps = make_in_maps(x, w_qkv, w_proj)
    res = run_bass_kernel_spmd(
        nc, in_maps, core_ids=list(range(N_CORES)), trace=_trace, tmpdir=_tmpdir
    )
    out = np.empty((B, T, D), dtype=np.float32)
    for b in range(B):
        out[b] = res.results[2 * b]["y"] + res.results[2 * b + 1]["y"]
    if _trace:
        kernel._last_results = res
    return out


# revision 8
# speedup vs baseline: 1.1947x; 1.0301x over previous
"""Causal multi-head attention (B=4, T=2048, D=1024, H=16) on 8 trn2 cores.

Sharding: core c -> (batch b = c//2, head-group g = c%2) -> 8 heads/core.

Per-core schedule (ACT-engine/exp is the roofline):
  - qg-outer loops; the 4 head pairs are processed as 2 blocks of TWO
    INTERLEAVED STREAMS so two independent exp->AV->QK dependency chains
    alternate on ACT and hide each other's semaphore latency.
  - QKV projections and the output projection are emitted as fine-grained
    "filler" interleaved into the attention chunk stream.
  - exp is one merged two-head instruction per k-chunk [128, 2, 512-off].
  - V carries 64 ones-columns so the AV matmul materializes the softmax
    denominator replicated across PSUM partitions 64..127; normalization is
    tensor_copy + reciprocal_approx_fast + two fused multiply-evictions.
  - causal mask applied by gpsimd affine_select directly on ex.
"""

import numpy as np
import ml_dtypes

import concourse.bass as bass  # noqa: F401  (bass types via bacc)
import concourse.bacc as bacc
import concourse.mybir as mybir
import concourse.tile as tile
from concourse.bass_utils import run_bass_kernel_spmd

B, T, D = 4, 2048, 1024
H, DH = 16, 64
N_CORES = 8
HPC = 8      # heads per core
PAIRS = HPC // 2
BF = mybir.dt.bfloat16
F32 = mybir.dt.float32
BF_NP = ml_dtypes.bfloat16

TQ = 512     # q block (free dim)
TK = 128     # k block (partition dim)
NQG = T // TQ
NKC = T // TK


def build_nc():
    nc = bacc.Bacc(
        "TRN2",
        target_bir_lowering=False,
        debug=False,
        enable_asserts=True,
        num_devices=N_CORES,
    )
    xT = nc.dram_tensor("xT", [D, T], BF, kind="ExternalInput")
    wq = nc.dram_tensor("wq", [D, 512], BF, kind="ExternalInput")
    wk = nc.dram_tensor("wk", [D, 512], BF, kind="ExternalInput")
    wv = nc.dram_tensor("wv", [D, 512], BF, kind="ExternalInput")
    wp = nc.dram_tensor("wp", [512, D], BF, kind="ExternalInput")
    y = nc.dram_tensor("y", [T, D], F32, kind="ExternalOutput")

    with tile.TileContext(nc) as tc:
        with (
            tc.tile_pool(name="pers", bufs=1) as pers,
            tc.tile_pool(name="work", bufs=1) as work,
            tc.tile_pool(name="ps", bufs=1, space="PSUM") as pp,
        ):
            # ---- persistent SBUF ----
            xT_sb = pers.tile([128, 8, T], BF, tag="xT", name="xT_sb")
            wq_sb = pers.tile([128, 8, 512], BF, tag="wq", name="wq_sb")
            wk_sb = pers.tile([128, 8, 512], BF, tag="wk", name="wk_sb")
            wv_sb = pers.tile([128, 8, 512], BF, tag="wv", name="wv_sb")
            wp_sb = pers.tile([128, 4, D], BF, tag="wp", name="wp_sb")
            # V in token-major layout; cols 64..127 are ones so the AV
            # matmul writes the denominator to psO partitions 64..127.
            vext = pers.tile([128, NKC, HPC, 128], BF, tag="vext", name="vext")
            qt_all = pers.tile([128, PAIRS, T], BF, tag="qt", name="qt_all")
            kt_all = pers.tile([128, PAIRS, T], BF, tag="kt", name="kt_all")
            # normalized attention outputs, d-major: [pair-chan, pair, tok]
            outT = pers.tile([128, PAIRS, T], BF, tag="outT", name="outT")

            # ---- loads (ordered so qg0 pair0/1 QK^T deps land first) ----
            for dc in range(8):
                nc.sync.dma_start(wq_sb[:, dc, :], wq[dc * 128:(dc + 1) * 128, :])
                nc.sync.dma_start(wk_sb[:, dc, :], wk[dc * 128:(dc + 1) * 128, :])
                nc.sync.dma_start(xT_sb[:, dc, 0:512], xT[dc * 128:(dc + 1) * 128, 0:512])
            for dc in range(8):
                nc.sync.dma_start(wv_sb[:, dc, :], wv[dc * 128:(dc + 1) * 128, :])
            for dc in range(8):
                nc.sync.dma_start(xT_sb[:, dc, 512:T], xT[dc * 128:(dc + 1) * 128, 512:T])
            for cc in range(4):
                nc.sync.dma_start(wp_sb[:, cc, :], wp[cc * 128:(cc + 1) * 128, :])
            nc.gpsimd.memset(vext[:, :, :, 64:128], 1.0)

            # ---- filler machinery ----
            filler = []
            fptr = [0]

            def pump(n=1):
                while n > 0 and fptr[0] < len(filler):
                    filler[fptr[0]]()
                    fptr[0] += 1
                    n -= 1

            def drain_until(idx):
                while fptr[0] <= idx:
                    if fptr[0] >= len(filler):
                        return
                    filler[fptr[0]]()
                    fptr[0] += 1

            # V tile t covers token chunks 2t, 2t+1 (all heads)
            def v_tile_closures(t):
                st = {}

                def mm(half, dlo):
                    if half == 0 and dlo == 0:
                        st["ps"] = pp.tile([128, 1024], F32, tag="sc", bufs=2,
                                           name="ps_v")
                    tk = 2 * t + half
                    for dc in range(dlo, dlo + 4):
                        nc.tensor.matmul(
                            st["ps"][:, half * 512:(half + 1) * 512],
                            xT_sb[:, dc, tk * 128:(tk + 1) * 128],
                            wv_sb[:, dc, :],
                            start=(dc == 0), stop=(dc == 7),
                        )

                def cast():
                    nc.vector.tensor_copy(
                        vext[:, 2 * t:2 * t + 2, :, 0:64],
                        st["ps"].rearrange("p (t h d) -> p t h d", t=2, d=64),
                    )

                return [lambda h=h, d=d: mm(h, d) for h in (0, 1) for d in (0, 4)] + [cast]

            # QK^T projection for (qg, pair): d-major Q^T/K^T of 512 tokens
            def qkt_closures(qg, pair):
                st = {}
                sl = slice(qg * TQ, (qg + 1) * TQ)

                def mm(w_sb, half, dlo):
                    if half == 0 and dlo == 0:
                        st["ps"] = pp.tile([128, 1024], F32, tag="sc", bufs=2,
                                           name="ps_qk")
                    for dc in range(dlo, dlo + 4):
                        nc.tensor.matmul(
                            st["ps"][:, half * 512:(half + 1) * 512],
                            w_sb[:, dc, pair * 128:(pair + 1) * 128],
                            xT_sb[:, dc, sl],
                            start=(dc == 0), stop=(dc == 7),
                        )

                def cast():
                    nc.vector.tensor_copy(qt_all[:, pair, sl], st["ps"][:, 0:512])
                    nc.vector.tensor_copy(kt_all[:, pair, sl], st["ps"][:, 512:1024])

                return [
                    lambda d=0: mm(wq_sb, 0, d),
                    lambda d=4: mm(wq_sb, 0, d),
                    lambda d=0: mm(wk_sb, 1, d),
                    lambda d=4: mm(wk_sb, 1, d),
                    cast,
                ]

            # output projection for one token chunk tk (needs outT of all pairs)
            def proj_closures(tk):
                st = {}

                def mm(nb):
                    if nb == 0:
                        st["ps"] = pp.tile([128, 1024], F32, tag="sc", bufs=2,
                                           name="ps_y")
                    for cc in range(4):
                        nc.tensor.matmul(
                            st["ps"][:, nb * 512:(nb + 1) * 512],
                            outT[:, cc, tk * 128:(tk + 1) * 128],
                            wp_sb[:, cc, nb * 512:(nb + 1) * 512],
                            start=(cc == 0), stop=(cc == 3),
                        )

                def evict():
                    y_sb = work.tile([128, 1024], F32, tag="ysb", bufs=2,
                                     name="y_sb")
                    nc.vector.tensor_copy(y_sb[:, :], st["ps"][:, :])
                    nc.sync.dma_start(y[tk * 128:(tk + 1) * 128, :], y_sb[:, :])

                return [lambda: mm(0), lambda: mm(1), evict]

            # prefix: only qg0 pair0+pair1 QK^T emitted directly; V chunks are
            # filler so the first exps start as early as possible
            for p in (0, 1):
                for cl in qkt_closures(0, p):
                    cl()

            # filler queue + markers[(qg,pair)] = last filler index that must
            # be emitted before attention on (qg, pair) starts; vmark[qg] =
            # last V-tile index needed by qg's AVs
            marker = {(0, 0): -1, (0, 1): -1}
            vmark = {}
            for t in (0, 1):
                filler.extend(v_tile_closures(t))
            vmark[0] = len(filler) - 1
            for p in (2, 3):
                filler.extend(qkt_closures(0, p))
                marker[(0, p)] = len(filler) - 1
            for qg in (1, 2, 3):
                for t in (2 * qg, 2 * qg + 1):
                    filler.extend(v_tile_closures(t))
                vmark[qg] = len(filler) - 1
                for p in range(4):
                    filler.extend(qkt_closures(qg, p))
                    marker[(qg, p)] = len(filler) - 1

            # ---- attention: 2 blocks of 2 interleaved pair-streams per qg ----
            # normalize work is dribbled into the next block's slots
            normq = []

            proj_pending = [None]

            for qg in range(NQG):
                noff = qg * (TQ // TK)
                kmax = (qg + 1) * (TQ // TK)

                def qk(pair, kc):
                    off = max(0, kc - noff) * TK
                    ps_s = pp.tile([128, 1024], F32, tag="sc", bufs=2,
                                   name="ps_s")
                    for h in (0, 1):
                        nc.tensor.matmul(
                            ps_s[:, h * 512 + off:(h + 1) * 512],
                            kt_all[h * 64:(h + 1) * 64, pair,
                                   kc * TK:(kc + 1) * TK],
                            qt_all[h * 64:(h + 1) * 64, pair,
                                   qg * TQ + off:(qg + 1) * TQ],
                            start=True, stop=True,
                        )
                    return ps_s

                def exp_mask(pair, kc, ps_s):
                    off = max(0, kc - noff) * TK
                    ex = work.tile([128, 2, 512], BF, tag="ex", bufs=8,
                                   name="ex")
                    s3 = ps_s.rearrange("p (h q) -> p h q", h=2)
                    nc.scalar.activation(
                        ex[:, :, off:512], s3[:, :, off:512],
                        mybir.ActivationFunctionType.Exp,
                    )
                    if kc >= noff:
                        # causal mask on the diagonal 128-col sub-block
                        nc.gpsimd.affine_select(
                            ex[:, :, off:off + TK],
                            ex[:, :, off:off + TK],
                            pattern=[[0, 2], [1, TK]],
                            compare_op=mybir.AluOpType.is_ge,
                            fill=0.0,
                            base=0,
                            channel_multiplier=-1,
                        )
                    return ex

                def av(pair, kc, ex, psO):
                    off = max(0, kc - noff) * TK
                    for h in (0, 1):
                        nc.tensor.matmul(
                            psO[:, h * 512 + off:(h + 1) * 512],
                            vext[:, kc, pair * 2 + h, :],
                            ex[:, h, off:512],
                            start=(kc == 0),
                            stop=(kc == kmax - 1),
                            skip_group_check=True,
                        )

                def finish_pair(pair, psO, qg=qg):
                    """Queue den copy + recip + multiply-evicts for dribbling
                    into the next block.  qg is bound at definition time: the
                    closures run during the NEXT qg iteration."""
                    st = {}

                    def c_den():
                        st["den"] = work.tile([64, 1024], F32, tag="den",
                                              bufs=2, name="den_sb")
                        nc.vector.tensor_copy(st["den"][:, :], psO[64:128, :])

                    def c_recip():
                        st["bcr"] = work.tile([64, 1024], F32, tag="bcr",
                                              bufs=2, name="bcr")
                        nc.vector.reciprocal_approx_fast(st["bcr"][:, :],
                                                         st["den"][:, :])

                    def c_mult(h):
                        nc.vector.tensor_mul(
                            outT[h * 64:(h + 1) * 64, pair,
                                 qg * TQ:(qg + 1) * TQ],
                            psO[0:64, h * 512:(h + 1) * 512],
                            st["bcr"][0:64, h * 512:(h + 1) * 512],
                        )

                    normq.extend([c_den, c_recip,
                                  lambda: c_mult(0), lambda: c_mult(1)])

                for blk in (0, 1):
                    pA, pB = 2 * blk, 2 * blk + 1
                    drain_until(max(marker[(qg, pA)], marker[(qg, pB)],
                                    vmark[qg]))
                    psO_A = pp.tile([128, 1024], F32, tag="o", bufs=2,
                                    name="psO_A")
                    psO_B = pp.tile([128, 1024], F32, tag="o", bufs=2,
                                    name="psO_B")
                    if blk == 0 and qg > 0:
                        proj_pending[0] = list(range((qg - 1) * 4, qg * 4))
                    sA = qk(pA, 0)
                    sB = qk(pB, 0)
                    eA = eB = None
                    for kc in range(kmax):
                        nA = qk(pA, kc + 1) if kc + 1 < kmax else None
                        nB = qk(pB, kc + 1) if kc + 1 < kmax else None
                        exA = exp_mask(pA, kc, sA)
                        exB = exp_mask(pB, kc, sB)
                        # dribble previous block's normalize work (DVE)
                        for _ in range(4):
                            if normq:
                                normq.pop(0)()
                        if not normq and proj_pending[0] is not None:
                            for tk in proj_pending[0]:
                                filler.extend(proj_closures(tk))
                            proj_pending[0] = None
                        # AVs lag one iteration so masks and norm dribble
                        # never sit ahead of them in an engine FIFO
                        if eA is not None:
                            av(pA, kc - 1, eA, psO_A)
                            av(pB, kc - 1, eB, psO_B)
                        if kc >= 2:
                            pump(2 if kc >= 4 else 4)
                        sA, sB = nA, nB
                        eA, eB = exA, exB
                    av(pA, kmax - 1, eA, psO_A)
                    av(pB, kmax - 1, eB, psO_B)
                    finish_pair(pA, psO_A)
                    finish_pair(pB, psO_B)

            # ---- tail: last normalize + last qg's projection ----
            while normq:
                normq.pop(0)()
            for tk in range(12, 16):
                filler.extend(proj_closures(tk))
            drain_until(len(filler) - 1)

    nc.compile()
    return nc


_NC_CACHE = None


def _get_nc():
    global _NC_CACHE
    if _NC_CACHE is None:
        _NC_CACHE = build_nc()
    return _NC_CACHE


def make_in_maps(x, w_qkv, w_proj):
    """Host-side sharding: core c -> (batch c//2, head-group c%2)."""
    scale = np.float32(1.0 / np.sqrt(DH))
    in_maps = []
    for c in range(N_CORES):
        b, g = divmod(c, 2)
        sl = slice(g * 512, (g + 1) * 512)
        xT = np.ascontiguousarray(x[b].T).astype(BF_NP)
        wq = (w_qkv[:, 0 * D:1 * D][:, sl] * scale).astype(BF_NP)
        wk = w_qkv[:, 1 * D:2 * D][:, sl].astype(BF_NP)
        wv = w_qkv[:, 2 * D:3 * D][:, sl].astype(BF_NP)
        wp = np.ascontiguousarray(w_proj[sl, :]).astype(BF_NP)
        in_maps.append({"xT": xT, "wq": wq, "wk": wk, "wv": wv, "wp": wp})
    return in_maps


def kernel(x, w_qkv, w_proj, _trace=False, _tmpdir=None):
    x = np.asarray(x, dtype=np.float32)
    w_qkv = np.asarray(w_qkv, dtype=np.float32)
    w_proj = np.asarray(w_proj, dtype=np.float32)
    nc = _get_nc()
    in_maps = make_in_maps(x, w_qkv, w_proj)
    res = run_bass_kernel_spmd(
        nc, in_maps, core_ids=list(range(N_CORES)), trace=_trace, tmpdir=_tmpdir
    )
    out = np.empty((B, T, D), dtype=np.float32)
    for b in range(B):
        out[b] = res.results[2 * b]["y"] + res.results[2 * b + 1]["y"]
    if _trace:
        kernel._last_results = res
    return out
